# revision 11
# baseline (speedup 1.0000x reference)
"""2-layer GAT (nn_GATNet) on 8 TRN2 NeuronCores — self-contained kernel.

Design (SPMD, one program on 8 cores, dst-node sharding 6250/core).

The runtime here charges ~40-75us per STATIC instruction, so the kernel is
built around hardware For_i loops with tiny bodies (~150 static instructions
total) and a matmul-free edge phase:

  A1 (For_i over 49 windows): t1_shard = x @ [W1 | W1 a1_src | W1 a1_dst]
     for the local dst shard ([6250,128] rows, 80 cols used, 512B rows),
     then AllGather -> t1 [50000,128] on every core.
  B1 (For_i over windows): K-binned dst-major edge layout. Each window = 128
     dst nodes (one per partition); bin k of dst d sits at gather slot
     k*128+d, so dma_gather (non-transpose) lands each dst's K bins on its
     own partition: g[d, k, :]. Bins are split into a lo section (src <
     32768, K1 bins) and a hi section (src-32768, K2 bins) because dma_gather
     indices are int16; hi gathers use a base-shifted table AP. Self loops
     are ordinary bins; pad bins gather row 0 and carry a -300 logit mask.
     Per window: ~7 dma_gathers (<=1024 idxs each, the ucode ring cap) +
     ~14 DVE/Act ops: logits = g[:,:,acol:acol+NH] + alpha_dst (free-dim
     broadcast from a direct window DMA of the local shard rows) + mask;
     ex = exp(leakyrelu); msg = g[:,:,0:HC]*ex; U/denom = free-axis reduce
     over k; agg = U/denom. No PE, no one-hot, no transposes.
  A2 (For_i): t2_shard = elu(h1) @ [W2 | ...] ([6250,64] rows, 42 used,
     256B rows), AllGather -> t2.
  B2: same edge phase with 1 head / 40 dims + log_softmax, writes the local
     [6250, 40] output shard; host concatenates shards.

Bin counts (K1, K2) are the max per-half in-degree over all windows/cores,
computed from the input (compile cache keyed on them).
"""
import numpy as np
import concourse.bass as bass
import concourse.bacc as bacc
import concourse.tile as tile
from concourse import mybir
from concourse import library_config
from concourse.bass import ds
from concourse.bass_utils import run_bass_kernel_spmd

P = 128
F32 = mybir.dt.float32
I16 = mybir.dt.int16
AF = mybir.ActivationFunctionType
OP = mybir.AluOpType

N_NODES = 50000
NC = 8
SHARD = N_NODES // NC          # 6250
NW = (SHARD + P - 1) // P      # 49 windows
PSH = NW * P + P               # padded shard rows (6400): dummy window 49
NWP = NW + 1                   # padded window count (50, even for pairing)
HALF = 32768                   # int16 index limit for dma_gather
MASKVAL = -300.0
W1O, W2O = 80, 42              # used cols of the two tables
T1W, T2W = 128, 64             # table row widths (512B / 256B)


def _fold_params(W1, a1_src, a1_dst, W2, a2_src, a2_dst):
    def fold(W, a):
        heads, od = a.shape
        return np.einsum("cho,ho->ch", W.reshape(W.shape[0], heads, od), a)
    W_ext1 = np.concatenate([W1, fold(W1, a1_src), fold(W1, a1_dst)], axis=1)
    W_ext2 = np.concatenate([W2, fold(W2, a2_src), fold(W2, a2_dst)], axis=1)
    return (np.ascontiguousarray(W_ext1, np.float32),
            np.ascontiguousarray(W_ext2, np.float32))


def _core_edges(src, dst, c):
    """Edges (incl. self loops) for core c: returns (esrc, eld)."""
    lo = c * SHARD
    m = (dst >= lo) & (dst < lo + SHARD)
    esrc = src[m]
    eld = dst[m] - lo
    selfn = np.arange(lo, lo + SHARD, dtype=np.int64)
    esrc = np.concatenate([esrc, selfn])
    eld = np.concatenate([eld, selfn - lo])
    return esrc, eld


def required_T(edge_index, N=None):
    """Global (K1, K2): max lo/hi bin count over all cores' windows."""
    src = np.asarray(edge_index[0]).astype(np.int64)
    dst = np.asarray(edge_index[1]).astype(np.int64)
    K1 = K2 = 0
    for c in range(NC):
        esrc, eld = _core_edges(src, dst, c)
        row = (esrc // SHARD) * PSH + esrc % SHARD
        hi = (row >= HALF).astype(np.int64)
        cnt = np.bincount(eld * 2 + hi, minlength=SHARD * 2).reshape(SHARD, 2)
        K1 = max(K1, int(cnt[:, 0].max()))
        K2 = max(K2, int(cnt[:, 1].max()))
    return (K1, K2)


def _gather_splits(K):
    """Split K*128 slots into dma_gather calls of <=1024 idxs (8 chunks)."""
    out = []
    left = K
    while left > 0:
        take = min(8, left)
        out.append(take * P)
        left -= take
    return out


def _wrap_blocks(flat, nidx_list):
    """Concat per-sub-block wrapped int16 idx layouts -> [128, total//16]."""
    cols = []
    off = 0
    for n in nidx_list:
        blk = flat[off:off + n]
        w = blk.reshape(-1, 16).T  # [16, n//16]
        cols.append(np.tile(w, (8, 1)))
        off += n
    return np.concatenate(cols, axis=1).astype(np.int16)


def _prep_edges(src, dst, K1, K2):
    """Per-core (idx_dram [128, NW*K*8] i16, mask_dram [128, NW*K] f32)."""
    K = K1 + K2
    splits = _gather_splits(K1) + _gather_splits(K2)
    per_core = []
    for c in range(NC):
        esrc, eld = _core_edges(src, dst, c)
        erow = (esrc // SHARD) * PSH + esrc % SHARD
        hi = (erow >= HALF).astype(np.int64)
        key = eld * 2 + hi
        order = np.argsort(key, kind="stable")
        ks = key[order]
        starts = np.searchsorted(ks, np.arange(SHARD * 2))
        rank = np.arange(len(ks)) - starts[ks]
        e_row = erow[order]
        e_ld = eld[order]
        e_hi = hi[order]
        w = e_ld >> 7
        d = e_ld & 127
        k = np.where(e_hi == 0, rank, K1 + rank)
        val = np.where(e_hi == 0, e_row, e_row - HALF)
        bins = np.zeros((NWP, K, P), np.int64)
        mask = np.full((NWP, P, K), MASKVAL, np.float32)
        bins[w, k, d] = val
        mask[w, d, k] = 0.0
        idx_cols = [_wrap_blocks(bins[wi].reshape(-1), splits)
                    for wi in range(NWP)]
        idx_dram = np.ascontiguousarray(np.concatenate(idx_cols, axis=1))
        mask_dram = np.ascontiguousarray(
            mask.transpose(1, 0, 2).reshape(P, NWP * K))
        per_core.append((idx_dram, mask_dram))
    return per_core


def build_kernel(K1, K2, reps=1):
    K = K1 + K2
    ICOL = K * 8          # idx cols per window
    splits1 = _gather_splits(K1)
    splits2 = _gather_splits(K2)

    nc = bacc.Bacc("TRN2", target_bir_lowering=False, debug=False,
                   num_swdge_queues=1)

    xT = nc.dram_tensor("xT", [P, PSH], F32, kind="ExternalInput")
    W_ext1 = nc.dram_tensor("W_ext1", [P, W1O], F32, kind="ExternalInput")
    W_ext2 = nc.dram_tensor("W_ext2", [64, W2O], F32, kind="ExternalInput")
    b1m = nc.dram_tensor("b1m", [P, 64], F32, kind="ExternalInput")
    b2m = nc.dram_tensor("b2m", [P, 40], F32, kind="ExternalInput")
    ident_in = nc.dram_tensor("ident_in", [P, P], F32, kind="ExternalInput")
    idx_in = nc.dram_tensor("idx_in", [P, NWP * ICOL], I16, kind="ExternalInput")
    mask_in = nc.dram_tensor("mask_in", [P, NWP * K], F32, kind="ExternalInput")
    out = nc.dram_tensor("out", [PSH, 40], F32, kind="ExternalOutput")

    t1_shard = nc.dram_tensor("t1_shard", [PSH, T1W], F32)
    t2_shard = nc.dram_tensor("t2_shard", [PSH, T2W], F32)
    t1 = nc.dram_tensor("t1", [NC * PSH, T1W], F32, addr_space="Shared")
    t2 = nc.dram_tensor("t2", [NC * PSH, T2W], F32, addr_space="Shared")

    with tile.TileContext(nc) as tc:
        nc.gpsimd.load_library(library_config.mlp)
        cp = tc.alloc_tile_pool(name="const", bufs=1)
        w1_sb = cp.tile([P, W1O], F32)
        nc.sync.dma_start(out=w1_sb[:], in_=W_ext1[:])
        w2_sb = cp.tile([64, W2O], F32)
        nc.sync.dma_start(out=w2_sb[:], in_=W_ext2[:])
        b1_sb = cp.tile([P, 64], F32)
        nc.sync.dma_start(out=b1_sb[:], in_=b1m[:])
        b2_sb = cp.tile([P, 40], F32)
        nc.sync.dma_start(out=b2_sb[:], in_=b2m[:])
        ident_sb = cp.tile([P, P], F32)
        nc.sync.dma_start(out=ident_sb[:], in_=ident_in[:])

        # two tile sets (double-buffering across loop iterations: set 1's
        # gathers/DMAs overlap set 0's compute, so cross-engine waits are
        # usually pre-satisfied when reached)
        wp = tc.alloc_tile_pool(name="work", bufs=1)
        pp = tc.alloc_tile_pool(name="ps", bufs=1, space="PSUM")
        S = []
        for s in range(2):
            t = {}
            t["xc"] = wp.tile([P, P], F32, name=f"xc{s}")
            t["hb"] = wp.tile([P, W1O], F32, name=f"hb{s}")
            t["gi"] = wp.tile([P, ICOL], I16, name=f"gi{s}")
            t["mk"] = wp.tile([P, K], F32, name=f"mk{s}")
            t["adg"] = wp.tile([P, 16], F32, name=f"adg{s}")
            t["g"] = wp.tile([P, K * T1W], F32, name=f"g{s}")
            t["ee"] = wp.tile([P, K, 8], F32, name=f"ee{s}")
            t["ex"] = wp.tile([P, K, 8], F32, name=f"ex{s}")
            t["msg"] = wp.tile([P, K, 64], F32, name=f"msg{s}")
            t["U"] = wp.tile([P, 64], F32, name=f"U{s}")
            t["den"] = wp.tile([P, 8], F32, name=f"den{s}")
            t["rec"] = wp.tile([P, 8], F32, name=f"rec{s}")
            t["agg"] = wp.tile([P, 64], F32, name=f"agg{s}")
            t["em"] = wp.tile([P, 64], F32, name=f"em{s}")
            t["h1"] = wp.tile([P, 64], F32, name=f"h1_{s}")
            t["h1c"] = wp.tile([P, 64], F32, name=f"h1c{s}")
            t["hT"] = wp.tile([64, P], F32, name=f"hT{s}")
            t["h2b"] = wp.tile([P, W2O], F32, name=f"h2b{s}")
            t["ob"] = wp.tile([P, 40], F32, name=f"ob{s}")
            t["ps1"] = pp.tile([P, W1O], F32, space="PSUM", name=f"ps1_{s}")
            t["psT"] = pp.tile([64, P], F32, space="PSUM", name=f"psT{s}")
            t["ps2"] = pp.tile([P, W2O], F32, space="PSUM", name=f"ps2_{s}")
            S.append(t)

        def a1_body(t, i_col, i_row, rows):
            nc.sync.dma_start(out=t["xc"][:, 0:rows], in_=xT[:, i_col])
            nc.tensor.matmul(out=t["ps1"][0:rows, :], lhsT=t["xc"][:, 0:rows],
                             rhs=w1_sb[:], start=True, stop=True)
            nc.scalar.activation(out=t["hb"][0:rows, :], in_=t["ps1"][0:rows, :],
                                 func=AF.Copy)
            nc.sync.dma_start(out=t1_shard[i_row, 0:W1O], in_=t["hb"][0:rows, :])

        def edge_body(t, i_row, i_idx, i_mask, rows, table, adtab, tshape,
                      usedw, NH, OD, post, load_im=True):
            """One window of the edge phase. i_* are ds() slices."""
            HC = NH * OD
            acol = usedw - 2 * NH
            gi, mk, adg = t["gi"], t["mk"], t["adg"]
            ee, ex, msg = t["ee"], t["ex"], t["msg"]
            U, den, rec, agg = t["U"], t["den"], t["rec"], t["agg"]
            gw = t["g"][:].rearrange("p (k w) -> p k w", w=tshape)[:, 0:K, :]
            if load_im:
                nc.sync.dma_start(out=gi[:], in_=idx_in[:, i_idx])
                nc.sync.dma_start(out=mk[:], in_=mask_in[:, i_mask])
            nc.sync.dma_start(out=adg[0:rows, 0:2 * NH],
                              in_=adtab[i_row, acol:acol + 2 * NH])
            off = 0
            coloff = 0
            for base, n_list in ((0, splits1), (1, splits2)):
                tab_ap = table[0:HALF, :] if base == 0 else table[HALF:NC * PSH, :]
                for n_idx in n_list:
                    nc.gpsimd.dma_gather(
                        out_ap=gw[:, off:off + n_idx // P, :],
                        in_ap=tab_ap,
                        idxs_ap=gi[:, coloff:coloff + n_idx // 16],
                        num_idxs=n_idx, num_idxs_reg=n_idx, elem_size=tshape,
                        queue_num=0)
                    off += n_idx // P
                    coloff += n_idx // 16
            # logits: e = alpha_src[slot] + alpha_dst[d] + mask
            nc.vector.tensor_add(
                out=ee[:, :, 0:NH], in0=gw[:, :, acol:acol + NH],
                in1=adg[:, None, NH:2 * NH].to_broadcast([P, K, NH]))
            nc.vector.tensor_add(
                out=ee[:, :, 0:NH], in0=ee[:, :, 0:NH],
                in1=mk[:, :, None].to_broadcast([P, K, NH]))
            nc.vector.scalar_tensor_tensor(
                out=ee[:, :, 0:NH], in0=ee[:, :, 0:NH], scalar=0.2,
                in1=ee[:, :, 0:NH], op0=OP.mult, op1=OP.max)
            nc.scalar.activation(out=ex[:, :, 0:NH], in_=ee[:, :, 0:NH],
                                 func=AF.Exp)
            # msg = h[slot] * ex ; U/den = reduce over k ; agg = U/den
            nc.vector.tensor_tensor(
                out=msg[:, :, 0:HC].rearrange("p k (h o) -> p k h o", o=OD),
                in0=gw[:, :, 0:HC].rearrange("p k (h o) -> p k h o", o=OD),
                in1=ex[:, :, 0:NH, None].to_broadcast([P, K, NH, OD]),
                op=OP.mult)
            nc.vector.reduce_sum(
                out=U[:, 0:HC, None],
                in_=msg[:, :, 0:HC].rearrange("p k f -> p f k"),
                axis=mybir.AxisListType.X)
            nc.vector.reduce_sum(
                out=den[:, 0:NH, None],
                in_=ex[:, :, 0:NH].rearrange("p k h -> p h k"),
                axis=mybir.AxisListType.X)
            nc.vector.reciprocal(rec[:, 0:NH], den[:, 0:NH])
            nc.vector.tensor_tensor(
                out=agg[:, 0:HC].rearrange("p (h o) -> p h o", o=OD),
                in0=U[:, 0:HC].rearrange("p (h o) -> p h o", o=OD),
                in1=rec[:, 0:NH, None].to_broadcast([P, NH, OD]), op=OP.mult)
            post(t, rows)

        def post1(i_h1):
            def post(t, rows):
                agg, em, h1 = t["agg"], t["em"], t["h1"]
                nc.vector.tensor_add(out=agg[:, 0:64], in0=agg[:, 0:64],
                                     in1=b1_sb[:])
                nc.scalar.activation(out=em[:], in_=agg[:, 0:64], func=AF.Exp)
                nc.vector.tensor_scalar(out=em[:], in0=em[:], scalar1=-1.0,
                                        scalar2=0.0, op0=OP.add, op1=OP.min)
                nc.vector.scalar_tensor_tensor(
                    out=h1[:], in0=agg[:, 0:64], scalar=0.0, in1=em[:],
                    op0=OP.max, op1=OP.add)
                # fused layer-2 row computation: t2_shard = h1 @ W_ext2
                nc.tensor.transpose(out=t["psT"][:], in_=h1[:],
                                    identity=ident_sb[:])
                nc.scalar.activation(out=t["hT"][:], in_=t["psT"][:],
                                     func=AF.Copy)
                nc.tensor.matmul(out=t["ps2"][:], lhsT=t["hT"][:], rhs=w2_sb[:],
                                 start=True, stop=True)
                nc.scalar.activation(out=t["h2b"][:], in_=t["ps2"][:],
                                     func=AF.Copy)
                nc.sync.dma_start(out=t2_shard[i_h1, 0:W2O],
                                  in_=t["h2b"][0:rows, :])
            return post

        def post2(i_out):
            def post(t, rows):
                agg, em, ob = t["agg"], t["em"], t["ob"]
                den, rec = t["den"], t["rec"]
                nc.vector.tensor_add(out=em[:, 0:40], in0=agg[:, 0:40],
                                     in1=b2_sb[:])
                nc.scalar.activation(out=ob[:], in_=em[:, 0:40], func=AF.Exp)
                nc.vector.reduce_sum(out=den[:, 1:2, None], in_=ob[:, None, :],
                                     axis=mybir.AxisListType.X)
                nc.scalar.activation(out=rec[:, 0:1], in_=den[:, 1:2],
                                     func=AF.Ln)
                nc.vector.tensor_sub(out=ob[:], in0=em[:, 0:40],
                                     in1=rec[:, 0:1].to_broadcast([P, 40]))
                nc.sync.dma_start(out=out[i_out, :], in_=ob[0:rows, :])
            return post

        for rep in range(reps):
            # ---- A1 ----
            with tc.For_i(0, NWP * P, 2 * P) as i:
                a1_body(S[0], ds(i, P), ds(i, P), P)
                a1_body(S[1], ds(i + P, P), ds(i + P, P), P)
            nc.gpsimd.collective_compute(
                "AllGather", OP.bypass, replica_groups=[list(range(NC))],
                ins=[t1_shard[:]], outs=[t1[:]])

            # ---- B1 (layer-2 row compute fused into post1) ----
            with tc.For_i(0, NWP, 2) as i:
                edge_body(S[0], ds(i * P, P), ds(i * ICOL, ICOL), ds(i * K, K),
                          P, t1, t1_shard, T1W, W1O, 8, 8, post1(ds(i * P, P)))
                edge_body(S[1], ds(i * P + P, P), ds(i * ICOL + ICOL, ICOL),
                          ds(i * K + K, K), P, t1, t1_shard, T1W, W1O, 8, 8,
                          post1(ds(i * P + P, P)))
            nc.gpsimd.collective_compute(
                "AllGather", OP.bypass, replica_groups=[list(range(NC))],
                ins=[t2_shard[:]], outs=[t2[:]])

            # ---- B2 ----
            with tc.For_i(0, NWP, 2) as i:
                edge_body(S[0], ds(i * P, P), ds(i * ICOL, ICOL), ds(i * K, K),
                          P, t2, t2_shard, T2W, W2O, 1, 40, post2(ds(i * P, P)))
                edge_body(S[1], ds(i * P + P, P), ds(i * ICOL + ICOL, ICOL),
                          ds(i * K + K, K), P, t2, t2_shard, T2W, W2O, 1, 40,
                          post2(ds(i * P + P, P)))

        pp.release()
        wp.release()
        cp.release()

    nc.compile()
    return nc


_CACHE = {}


def _get_nc(T, reps=1):
    key = (T, reps)
    if key not in _CACHE:
        K1, K2 = T
        _CACHE[key] = build_kernel(K1, K2, reps=reps)
    return _CACHE[key]


def make_in_maps(x, edge_index, W1, a1_src, a1_dst, b1, W2, a2_src, a2_dst, b2,
                 T, N=None):
    K1, K2 = T
    W_ext1, W_ext2 = _fold_params(W1, a1_src, a1_dst, W2, a2_src, a2_dst)
    src = np.asarray(edge_index[0]).astype(np.int64)
    dst = np.asarray(edge_index[1]).astype(np.int64)
    per_core = _prep_edges(src, dst, K1, K2)
    xTf = np.ascontiguousarray(np.asarray(x, np.float32).T)
    shared = {
        "W_ext1": W_ext1, "W_ext2": W_ext2,
        "b1m": np.tile(np.asarray(b1, np.float32)[None, :], (P, 1)),
        "b2m": np.tile(np.asarray(b2, np.float32)[None, :], (P, 1)),
        "ident_in": np.eye(P, dtype=np.float32),
    }
    maps = []
    for c, (ix, mk) in enumerate(per_core):
        xp = np.zeros((P, PSH), np.float32)
        xp[:, 0:SHARD] = xTf[:, c * SHARD:(c + 1) * SHARD]
        maps.append(dict(shared, idx_in=ix, mask_in=mk, xT=xp))
    return maps


def kernel(x, edge_index, W1, a1_src, a1_dst, b1, W2, a2_src, a2_dst, b2,
           reps=1, nc_override=None):
    x = np.asarray(x, np.float32)
    edge_index = np.asarray(edge_index)
    args = [np.asarray(a, np.float32) for a in
            (W1, a1_src, a1_dst, b1, W2, a2_src, a2_dst, b2)]
    T = required_T(edge_index)
    in_maps = make_in_maps(x, edge_index, *args, T)
    nc = nc_override if nc_override is not None else _get_nc(T, reps)
    res = run_bass_kernel_spmd(nc, in_maps, list(range(NC)))
    return np.concatenate([res.results[c]["out"][0:SHARD] for c in range(NC)],
                          axis=0)


# revision 13
# speedup vs baseline: 1.4940x; 1.4940x over previous
"""2-layer GAT (nn_GATNet) on 8 TRN2 NeuronCores — self-contained kernel.

Design (SPMD, one program on 8 cores, dst-node sharding 6250/core).

The runtime here charges ~40-75us per STATIC instruction, so the kernel is
built around hardware For_i loops with tiny bodies (~150 static instructions
total) and a matmul-free edge phase:

  A1 (For_i over 49 windows): t1_shard = x @ [W1 | W1 a1_src | W1 a1_dst]
     for the local dst shard ([6250,128] rows, 80 cols used, 512B rows),
     then AllGather -> t1 [50000,128] on every core.
  B1 (For_i over windows): K-binned dst-major edge layout. Each window = 128
     dst nodes (one per partition); bin k of dst d sits at gather slot
     k*128+d, so dma_gather (non-transpose) lands each dst's K bins on its
     own partition: g[d, k, :]. Bins are split into a lo section (src <
     32768, K1 bins) and a hi section (src-32768, K2 bins) because dma_gather
     indices are int16; hi gathers use a base-shifted table AP. Self loops
     are ordinary bins; pad bins gather row 0 and carry a -300 logit mask.
     Per window: ~7 dma_gathers (<=1024 idxs each, the ucode ring cap) +
     ~14 DVE/Act ops: logits = g[:,:,acol:acol+NH] + alpha_dst (free-dim
     broadcast from a direct window DMA of the local shard rows) + mask;
     ex = exp(leakyrelu); msg = g[:,:,0:HC]*ex; U/denom = free-axis reduce
     over k; agg = U/denom. No PE, no one-hot, no transposes.
  A2 (For_i): t2_shard = elu(h1) @ [W2 | ...] ([6250,64] rows, 42 used,
     256B rows), AllGather -> t2.
  B2: same edge phase with 1 head / 40 dims + log_softmax, writes the local
     [6250, 40] output shard; host concatenates shards.

Bin counts (K1, K2) are the max per-half in-degree over all windows/cores,
computed from the input (compile cache keyed on them).
"""
import numpy as np
import concourse.bass as bass
import concourse.bacc as bacc
import concourse.tile as tile
from concourse import mybir
from concourse import library_config
from concourse.bass import ds
from concourse.bass_utils import run_bass_kernel_spmd

P = 128
F32 = mybir.dt.float32
I16 = mybir.dt.int16
AF = mybir.ActivationFunctionType
OP = mybir.AluOpType

N_NODES = 50000
NC = 8
SHARD = N_NODES // NC          # 6250
NW = (SHARD + P - 1) // P      # 49 windows
PSH = NW * P + P               # padded shard rows (6400): dummy window 49
NWP = NW + 1                   # padded window count (50, even for pairing)
HALF = 32768                   # int16 index limit for dma_gather
MASKVAL = -80.0   # pad-row alpha: exp(leaky(-80+e)) <= ~5e-7 yet never
                  # underflows denom to 0 even when doubled (alpha_src+alpha_dst)
W1O, W2O = 80, 42              # used cols of the two tables
T1W, T2W = 128, 64             # table row widths (512B / 256B)


def _fold_params(W1, a1_src, a1_dst, W2, a2_src, a2_dst):
    def fold(W, a):
        heads, od = a.shape
        return np.einsum("cho,ho->ch", W.reshape(W.shape[0], heads, od), a)
    W_ext1 = np.concatenate([W1, fold(W1, a1_src), fold(W1, a1_dst)], axis=1)
    W_ext2 = np.concatenate([W2, fold(W2, a2_src), fold(W2, a2_dst)], axis=1)
    return (np.ascontiguousarray(W_ext1, np.float32),
            np.ascontiguousarray(W_ext2, np.float32))


def _core_edges(src, dst, c):
    """Edges (incl. self loops) for core c: returns (esrc, eld)."""
    lo = c * SHARD
    m = (dst >= lo) & (dst < lo + SHARD)
    esrc = src[m]
    eld = dst[m] - lo
    selfn = np.arange(lo, lo + SHARD, dtype=np.int64)
    esrc = np.concatenate([esrc, selfn])
    eld = np.concatenate([eld, selfn - lo])
    return esrc, eld


def required_T(edge_index, N=None):
    """Global (K1, K2): max lo/hi bin count over all cores' windows."""
    src = np.asarray(edge_index[0]).astype(np.int64)
    dst = np.asarray(edge_index[1]).astype(np.int64)
    K1 = K2 = 0
    for c in range(NC):
        esrc, eld = _core_edges(src, dst, c)
        row = (esrc // SHARD) * PSH + esrc % SHARD
        hi = (row >= HALF).astype(np.int64)
        cnt = np.bincount(eld * 2 + hi, minlength=SHARD * 2).reshape(SHARD, 2)
        K1 = max(K1, int(cnt[:, 0].max()))
        K2 = max(K2, int(cnt[:, 1].max()))
    return (K1, K2)


def _gather_splits(K):
    """Split K*128 slots into dma_gather calls of <=1024 idxs (8 chunks)."""
    out = []
    left = K
    while left > 0:
        take = min(8, left)
        out.append(take * P)
        left -= take
    return out


def _wrap_blocks(flat, nidx_list):
    """Concat per-sub-block wrapped int16 idx layouts -> [128, total//16]."""
    cols = []
    off = 0
    for n in nidx_list:
        blk = flat[off:off + n]
        w = blk.reshape(-1, 16).T  # [16, n//16]
        cols.append(np.tile(w, (8, 1)))
        off += n
    return np.concatenate(cols, axis=1).astype(np.int16)


def _prep_edges(src, dst, K1, K2):
    """Per-core (idx_dram [128, NW*K*8] i16, mask_dram [128, NW*K] f32)."""
    K = K1 + K2
    splits = _gather_splits(K1) + _gather_splits(K2)
    per_core = []
    for c in range(NC):
        esrc, eld = _core_edges(src, dst, c)
        erow = (esrc // SHARD) * PSH + esrc % SHARD
        hi = (erow >= HALF).astype(np.int64)
        key = eld * 2 + hi
        order = np.argsort(key, kind="stable")
        ks = key[order]
        starts = np.searchsorted(ks, np.arange(SHARD * 2))
        rank = np.arange(len(ks)) - starts[ks]
        e_row = erow[order]
        e_ld = eld[order]
        e_hi = hi[order]
        w = e_ld >> 7
        d = e_ld & 127
        k = np.where(e_hi == 0, rank, K1 + rank)
        val = np.where(e_hi == 0, e_row, e_row - HALF)
        # pad bins gather the poison row (alpha_src = -300 -> exp ~ 0):
        # lo: local pad row PSH-1 of core 0; hi: core 7's pad row.
        bins = np.empty((NWP, K, P), np.int64)
        bins[:, 0:K1, :] = PSH - 1
        bins[:, K1:K, :] = NC * PSH - 1 - HALF
        bins[w, k, d] = val
        idx_cols = [_wrap_blocks(bins[wi].reshape(-1), splits)
                    for wi in range(NWP)]
        idx_dram = np.ascontiguousarray(np.concatenate(idx_cols, axis=1))
        per_core.append(idx_dram)
    return per_core


def build_kernel(K1, K2, reps=1):
    K = K1 + K2
    ICOL = K * 8          # idx cols per window
    splits1 = _gather_splits(K1)
    splits2 = _gather_splits(K2)

    nc = bacc.Bacc("TRN2", target_bir_lowering=False, debug=False,
                   num_swdge_queues=1)

    xT = nc.dram_tensor("xT", [P, PSH], F32, kind="ExternalInput")
    W_ext1 = nc.dram_tensor("W_ext1", [P, W1O], F32, kind="ExternalInput")
    W_ext2 = nc.dram_tensor("W_ext2", [64, W2O], F32, kind="ExternalInput")
    b1m = nc.dram_tensor("b1m", [P, 64], F32, kind="ExternalInput")
    b2m = nc.dram_tensor("b2m", [P, 40], F32, kind="ExternalInput")
    ident_in = nc.dram_tensor("ident_in", [P, P], F32, kind="ExternalInput")
    idx_in = nc.dram_tensor("idx_in", [P, NWP * ICOL], I16, kind="ExternalInput")
    out = nc.dram_tensor("out", [PSH, 40], F32, kind="ExternalOutput")

    t1_shard = nc.dram_tensor("t1_shard", [PSH, T1W], F32)
    t2_shard = nc.dram_tensor("t2_shard", [PSH, T2W], F32)
    t1 = nc.dram_tensor("t1", [NC * PSH, T1W], F32, addr_space="Shared")
    t2 = nc.dram_tensor("t2", [NC * PSH, T2W], F32, addr_space="Shared")

    with tile.TileContext(nc) as tc:
        nc.gpsimd.load_library(library_config.mlp)
        cp = tc.alloc_tile_pool(name="const", bufs=1)
        w1_sb = cp.tile([P, W1O], F32)
        nc.sync.dma_start(out=w1_sb[:], in_=W_ext1[:])
        w2_sb = cp.tile([64, W2O], F32)
        nc.sync.dma_start(out=w2_sb[:], in_=W_ext2[:])
        b1_sb = cp.tile([P, 64], F32)
        nc.sync.dma_start(out=b1_sb[:], in_=b1m[:])
        b2_sb = cp.tile([P, 40], F32)
        nc.sync.dma_start(out=b2_sb[:], in_=b2m[:])
        ident_sb = cp.tile([P, P], F32)
        nc.sync.dma_start(out=ident_sb[:], in_=ident_in[:])
        pois = cp.tile([1, 16], F32)
        nc.vector.memset(pois[:], MASKVAL)

        # two tile sets (double-buffering across loop iterations: set 1's
        # gathers/DMAs overlap set 0's compute, so cross-engine waits are
        # usually pre-satisfied when reached)
        wp = tc.alloc_tile_pool(name="work", bufs=1)
        pp = tc.alloc_tile_pool(name="ps", bufs=1, space="PSUM")
        S = []
        for s in range(2):
            t = {}
            t["xc"] = wp.tile([P, P], F32, name=f"xc{s}")
            t["hb"] = wp.tile([P, W1O], F32, name=f"hb{s}")
            t["gi"] = wp.tile([P, ICOL], I16, name=f"gi{s}")
            t["adg"] = wp.tile([P, 16], F32, name=f"adg{s}")
            t["g"] = wp.tile([P, K * T1W], F32, name=f"g{s}")
            t["ee"] = wp.tile([P, K, 8], F32, name=f"ee{s}")
            t["ex"] = wp.tile([P, K, 8], F32, name=f"ex{s}")
            t["msg"] = wp.tile([P, K, 64], F32, name=f"msg{s}")
            t["U"] = wp.tile([P, 64], F32, name=f"U{s}")
            t["den"] = wp.tile([P, 8], F32, name=f"den{s}")
            t["rec"] = wp.tile([P, 8], F32, name=f"rec{s}")
            t["agg"] = wp.tile([P, 64], F32, name=f"agg{s}")
            t["em"] = wp.tile([P, 64], F32, name=f"em{s}")
            t["h1"] = wp.tile([P, 64], F32, name=f"h1_{s}")
            t["h1c"] = wp.tile([P, 64], F32, name=f"h1c{s}")
            t["hT"] = wp.tile([64, P], F32, name=f"hT{s}")
            t["h2b"] = wp.tile([P, W2O], F32, name=f"h2b{s}")
            t["ob"] = wp.tile([P, 40], F32, name=f"ob{s}")
            t["ps1"] = pp.tile([P, W1O], F32, space="PSUM", name=f"ps1_{s}")
            t["psT"] = pp.tile([64, P], F32, space="PSUM", name=f"psT{s}")
            t["ps2"] = pp.tile([P, W2O], F32, space="PSUM", name=f"ps2_{s}")
            S.append(t)

        def a1_body(t, i_col, i_row, rows):
            nc.sync.dma_start(out=t["xc"][:, 0:rows], in_=xT[:, i_col])
            nc.tensor.matmul(out=t["ps1"][0:rows, :], lhsT=t["xc"][:, 0:rows],
                             rhs=w1_sb[:], start=True, stop=True)
            nc.scalar.activation(out=t["hb"][0:rows, :], in_=t["ps1"][0:rows, :],
                                 func=AF.Copy)
            nc.sync.dma_start(out=t1_shard[i_row, 0:W1O], in_=t["hb"][0:rows, :])

        def edge_body(t, i_row, i_idx, rows, table, adtab, tshape,
                      usedw, NH, OD, post):
            """One window of the edge phase. i_* are ds() slices."""
            HC = NH * OD
            acol = usedw - 2 * NH
            gi, adg = t["gi"], t["adg"]
            ee, ex, msg = t["ee"], t["ex"], t["msg"]
            U, den, rec, agg = t["U"], t["den"], t["rec"], t["agg"]
            gw = t["g"][:].rearrange("p (k w) -> p k w", w=tshape)[:, 0:K, :]
            nc.sync.dma_start(out=gi[:], in_=idx_in[:, i_idx])
            nc.sync.dma_start(out=adg[0:rows, 0:2 * NH],
                              in_=adtab[i_row, acol:acol + 2 * NH])
            off = 0
            coloff = 0
            for base, n_list in ((0, splits1), (1, splits2)):
                tab_ap = table[0:HALF, :] if base == 0 else table[HALF:NC * PSH, :]
                for n_idx in n_list:
                    nc.gpsimd.dma_gather(
                        out_ap=gw[:, off:off + n_idx // P, :],
                        in_ap=tab_ap,
                        idxs_ap=gi[:, coloff:coloff + n_idx // 16],
                        num_idxs=n_idx, num_idxs_reg=n_idx, elem_size=tshape,
                        queue_num=0)
                    off += n_idx // P
                    coloff += n_idx // 16
            # logits: e = alpha_src[slot] + alpha_dst[d] + mask
            nc.vector.tensor_add(
                out=ee[:, :, 0:NH], in0=gw[:, :, acol:acol + NH],
                in1=adg[:, None, NH:2 * NH].to_broadcast([P, K, NH]))
            nc.vector.scalar_tensor_tensor(
                out=ee[:, :, 0:NH], in0=ee[:, :, 0:NH], scalar=0.2,
                in1=ee[:, :, 0:NH], op0=OP.mult, op1=OP.max)
            nc.scalar.activation(out=ex[:, :, 0:NH], in_=ee[:, :, 0:NH],
                                 func=AF.Exp)
            # msg = h[slot] * ex ; U/den = reduce over k ; agg = U/den
            nc.vector.tensor_tensor(
                out=msg[:, :, 0:HC].rearrange("p k (h o) -> p k h o", o=OD),
                in0=gw[:, :, 0:HC].rearrange("p k (h o) -> p k h o", o=OD),
                in1=ex[:, :, 0:NH, None].to_broadcast([P, K, NH, OD]),
                op=OP.mult)
            nc.vector.reduce_sum(
                out=U[:, 0:HC, None],
                in_=msg[:, :, 0:HC].rearrange("p k f -> p f k"),
                axis=mybir.AxisListType.X)
            nc.vector.reduce_sum(
                out=den[:, 0:NH, None],
                in_=ex[:, :, 0:NH].rearrange("p k h -> p h k"),
                axis=mybir.AxisListType.X)
            nc.vector.reciprocal(rec[:, 0:NH], den[:, 0:NH])
            nc.vector.tensor_tensor(
                out=agg[:, 0:HC].rearrange("p (h o) -> p h o", o=OD),
                in0=U[:, 0:HC].rearrange("p (h o) -> p h o", o=OD),
                in1=rec[:, 0:NH, None].to_broadcast([P, NH, OD]), op=OP.mult)
            post(t, rows)

        def post1(i_h1):
            def post(t, rows):
                agg, em, h1 = t["agg"], t["em"], t["h1"]
                nc.vector.tensor_add(out=agg[:, 0:64], in0=agg[:, 0:64],
                                     in1=b1_sb[:])
                nc.scalar.activation(out=em[:], in_=agg[:, 0:64], func=AF.Exp)
                nc.vector.tensor_scalar(out=em[:], in0=em[:], scalar1=-1.0,
                                        scalar2=0.0, op0=OP.add, op1=OP.min)
                nc.vector.scalar_tensor_tensor(
                    out=h1[:], in0=agg[:, 0:64], scalar=0.0, in1=em[:],
                    op0=OP.max, op1=OP.add)
                # fused layer-2 row computation: t2_shard = h1 @ W_ext2
                nc.tensor.transpose(out=t["psT"][:], in_=h1[:],
                                    identity=ident_sb[:])
                nc.scalar.activation(out=t["hT"][:], in_=t["psT"][:],
                                     func=AF.Copy)
                nc.tensor.matmul(out=t["ps2"][:], lhsT=t["hT"][:], rhs=w2_sb[:],
                                 start=True, stop=True)
                nc.scalar.activation(out=t["h2b"][:], in_=t["ps2"][:],
                                     func=AF.Copy)
                nc.sync.dma_start(out=t2_shard[i_h1, 0:W2O],
                                  in_=t["h2b"][0:rows, :])
            return post

        def post2(i_out):
            def post(t, rows):
                agg, em, ob = t["agg"], t["em"], t["ob"]
                den, rec = t["den"], t["rec"]
                nc.vector.tensor_add(out=em[:, 0:40], in0=agg[:, 0:40],
                                     in1=b2_sb[:])
                nc.scalar.activation(out=ob[:], in_=em[:, 0:40], func=AF.Exp)
                nc.vector.reduce_sum(out=den[:, 1:2, None], in_=ob[:, None, :],
                                     axis=mybir.AxisListType.X)
                nc.scalar.activation(out=rec[:, 0:1], in_=den[:, 1:2],
                                     func=AF.Ln)
                nc.vector.tensor_sub(out=ob[:], in0=em[:, 0:40],
                                     in1=rec[:, 0:1].to_broadcast([P, 40]))
                nc.sync.dma_start(out=out[i_out, :], in_=ob[0:rows, :])
            return post

        for rep in range(reps):
            # ---- A1 ----
            with tc.For_i(0, NWP * P, 2 * P) as i:
                a1_body(S[0], ds(i, P), ds(i, P), P)
                a1_body(S[1], ds(i + P, P), ds(i + P, P), P)
            nc.sync.dma_start(out=t1_shard[PSH - 1:PSH, W1O - 16:W1O],
                              in_=pois[:, 0:16])
            nc.gpsimd.collective_compute(
                "AllGather", OP.bypass, replica_groups=[list(range(NC))],
                ins=[t1_shard[:]], outs=[t1[:]])

            # ---- B1 (layer-2 row compute fused into post1) ----
            with tc.For_i(0, NWP, 2) as i:
                edge_body(S[0], ds(i * P, P), ds(i * ICOL, ICOL),
                          P, t1, t1_shard, T1W, W1O, 8, 8, post1(ds(i * P, P)))
                edge_body(S[1], ds(i * P + P, P), ds(i * ICOL + ICOL, ICOL),
                          P, t1, t1_shard, T1W, W1O, 8, 8,
                          post1(ds(i * P + P, P)))
            nc.sync.dma_start(out=t2_shard[PSH - 1:PSH, W2O - 2:W2O],
                              in_=pois[:, 0:2])
            nc.gpsimd.collective_compute(
                "AllGather", OP.bypass, replica_groups=[list(range(NC))],
                ins=[t2_shard[:]], outs=[t2[:]])

            # ---- B2 ----
            with tc.For_i(0, NWP, 2) as i:
                edge_body(S[0], ds(i * P, P), ds(i * ICOL, ICOL),
                          P, t2, t2_shard, T2W, W2O, 1, 40, post2(ds(i * P, P)))
                edge_body(S[1], ds(i * P + P, P), ds(i * ICOL + ICOL, ICOL),
                          P, t2, t2_shard, T2W, W2O, 1, 40,
                          post2(ds(i * P + P, P)))

        pp.release()
        wp.release()
        cp.release()

    nc.compile()
    return nc


_CACHE = {}


def _get_nc(T, reps=1):
    key = (T, reps)
    if key not in _CACHE:
        K1, K2 = T
        _CACHE[key] = build_kernel(K1, K2, reps=reps)
    return _CACHE[key]


def make_in_maps(x, edge_index, W1, a1_src, a1_dst, b1, W2, a2_src, a2_dst, b2,
                 T, N=None):
    K1, K2 = T
    W_ext1, W_ext2 = _fold_params(W1, a1_src, a1_dst, W2, a2_src, a2_dst)
    src = np.asarray(edge_index[0]).astype(np.int64)
    dst = np.asarray(edge_index[1]).astype(np.int64)
    per_core = _prep_edges(src, dst, K1, K2)
    xTf = np.ascontiguousarray(np.asarray(x, np.float32).T)
    shared = {
        "W_ext1": W_ext1, "W_ext2": W_ext2,
        "b1m": np.tile(np.asarray(b1, np.float32)[None, :], (P, 1)),
        "b2m": np.tile(np.asarray(b2, np.float32)[None, :], (P, 1)),
        "ident_in": np.eye(P, dtype=np.float32),
    }
    maps = []
    for c, ix in enumerate(per_core):
        xp = np.zeros((P, PSH), np.float32)
        xp[:, 0:SHARD] = xTf[:, c * SHARD:(c + 1) * SHARD]
        maps.append(dict(shared, idx_in=ix, xT=xp))
    return maps


def kernel(x, edge_index, W1, a1_src, a1_dst, b1, W2, a2_src, a2_dst, b2,
           reps=1, nc_override=None):
    x = np.asarray(x, np.float32)
    edge_index = np.asarray(edge_index)
    args = [np.asarray(a, np.float32) for a in
            (W1, a1_src, a1_dst, b1, W2, a2_src, a2_dst, b2)]
    T = required_T(edge_index)
    in_maps = make_in_maps(x, edge_index, *args, T)
    nc = nc_override if nc_override is not None else _get_nc(T, reps)
    res = run_bass_kernel_spmd(nc, in_maps, list(range(NC)))
    return np.concatenate([res.results[c]["out"][0:SHARD] for c in range(NC)],
                          axis=0)


# revision 14
# speedup vs baseline: 2.6664x; 1.7847x over previous
"""2-layer GAT (nn_GATNet) on 8 TRN2 NeuronCores — self-contained kernel.

Design (SPMD, one program on 8 cores, dst-node sharding 6250/core).

The runtime here charges ~40-75us per STATIC instruction, so the kernel is
built around hardware For_i loops with tiny bodies (~150 static instructions
total) and a matmul-free edge phase:

  A1 (For_i over 49 windows): t1_shard = x @ [W1 | W1 a1_src | W1 a1_dst]
     for the local dst shard ([6250,128] rows, 80 cols used, 512B rows),
     then AllGather -> t1 [50000,128] on every core.
  B1 (For_i over windows): K-binned dst-major edge layout. Each window = 128
     dst nodes (one per partition); bin k of dst d sits at gather slot
     k*128+d, so dma_gather (non-transpose) lands each dst's K bins on its
     own partition: g[d, k, :]. Bins are split into a lo section (src <
     32768, K1 bins) and a hi section (src-32768, K2 bins) because dma_gather
     indices are int16; hi gathers use a base-shifted table AP. Self loops
     are ordinary bins; pad bins gather row 0 and carry a -300 logit mask.
     Per window: ~7 dma_gathers (<=1024 idxs each, the ucode ring cap) +
     ~14 DVE/Act ops: logits = g[:,:,acol:acol+NH] + alpha_dst (free-dim
     broadcast from a direct window DMA of the local shard rows) + mask;
     ex = exp(leakyrelu); msg = g[:,:,0:HC]*ex; U/denom = free-axis reduce
     over k; agg = U/denom. No PE, no one-hot, no transposes.
  A2 (For_i): t2_shard = elu(h1) @ [W2 | ...] ([6250,64] rows, 42 used,
     256B rows), AllGather -> t2.
  B2: same edge phase with 1 head / 40 dims + log_softmax, writes the local
     [6250, 40] output shard; host concatenates shards.

Bin counts (K1, K2) are the max per-half in-degree over all windows/cores,
computed from the input (compile cache keyed on them).
"""
import numpy as np
import concourse.bass as bass
import concourse.bacc as bacc
import concourse.tile as tile
from concourse import mybir
from concourse import library_config
from concourse.bass import ds
from concourse.bass_utils import run_bass_kernel_spmd

P = 128
F32 = mybir.dt.float32
I16 = mybir.dt.int16
AF = mybir.ActivationFunctionType
OP = mybir.AluOpType

N_NODES = 50000
NC = 8
SHARD = N_NODES // NC          # 6250
NW = (SHARD + P - 1) // P      # 49 windows
PSH = NW * P + P               # padded shard rows (6400): dummy window 49
NWP = NW + 1                   # padded window count (50, even for pairing)
HALF = 32768                   # int16 index limit for dma_gather
MASKVAL = -80.0   # pad-row alpha: exp(leaky(-80+e)) <= ~5e-7 yet never
                  # underflows denom to 0 even when doubled (alpha_src+alpha_dst)
W1O, W2O = 80, 42              # used cols of the two tables
T1W, T2W = 128, 64             # table row widths (512B / 256B)


def _fold_params(W1, a1_src, a1_dst, W2, a2_src, a2_dst):
    def fold(W, a):
        heads, od = a.shape
        return np.einsum("cho,ho->ch", W.reshape(W.shape[0], heads, od), a)
    W_ext1 = np.concatenate([W1, fold(W1, a1_src), fold(W1, a1_dst)], axis=1)
    W_ext2 = np.concatenate([W2, fold(W2, a2_src), fold(W2, a2_dst)], axis=1)
    return (np.ascontiguousarray(W_ext1, np.float32),
            np.ascontiguousarray(W_ext2, np.float32))


def _core_edges(src, dst, c):
    """Edges (incl. self loops) for core c: returns (esrc, eld)."""
    lo = c * SHARD
    m = (dst >= lo) & (dst < lo + SHARD)
    esrc = src[m]
    eld = dst[m] - lo
    selfn = np.arange(lo, lo + SHARD, dtype=np.int64)
    esrc = np.concatenate([esrc, selfn])
    eld = np.concatenate([eld, selfn - lo])
    return esrc, eld


def required_T(edge_index, N=None):
    """Global (K1, K2): max lo/hi bin count over all cores' windows."""
    src = np.asarray(edge_index[0]).astype(np.int64)
    dst = np.asarray(edge_index[1]).astype(np.int64)
    K1 = K2 = 0
    for c in range(NC):
        esrc, eld = _core_edges(src, dst, c)
        row = (esrc // SHARD) * PSH + esrc % SHARD
        hi = (row >= HALF).astype(np.int64)
        cnt = np.bincount(eld * 2 + hi, minlength=SHARD * 2).reshape(SHARD, 2)
        K1 = max(K1, int(cnt[:, 0].max()))
        K2 = max(K2, int(cnt[:, 1].max()))
    return (K1, K2)


def _gather_splits(K):
    """Split K*128 slots into dma_gather calls of <=1024 idxs (8 chunks)."""
    out = []
    left = K
    while left > 0:
        take = min(8, left)
        out.append(take * P)
        left -= take
    return out


def _wrap_blocks(flat, nidx_list):
    """Concat per-sub-block wrapped int16 idx layouts -> [128, total//16]."""
    cols = []
    off = 0
    for n in nidx_list:
        blk = flat[off:off + n]
        w = blk.reshape(-1, 16).T  # [16, n//16]
        cols.append(np.tile(w, (8, 1)))
        off += n
    return np.concatenate(cols, axis=1).astype(np.int16)


def _prep_edges(src, dst, K1, K2):
    """Per-core (idx_dram [128, NW*K*8] i16, mask_dram [128, NW*K] f32)."""
    K = K1 + K2
    splits = _gather_splits(K1) + _gather_splits(K2)
    per_core = []
    for c in range(NC):
        esrc, eld = _core_edges(src, dst, c)
        erow = (esrc // SHARD) * PSH + esrc % SHARD
        hi = (erow >= HALF).astype(np.int64)
        key = eld * 2 + hi
        order = np.argsort(key, kind="stable")
        ks = key[order]
        starts = np.searchsorted(ks, np.arange(SHARD * 2))
        rank = np.arange(len(ks)) - starts[ks]
        e_row = erow[order]
        e_ld = eld[order]
        e_hi = hi[order]
        w = e_ld >> 7
        d = e_ld & 127
        k = np.where(e_hi == 0, rank, K1 + rank)
        val = np.where(e_hi == 0, e_row, e_row - HALF)
        # pad bins gather the poison row (alpha_src = -300 -> exp ~ 0):
        # lo: local pad row PSH-1 of core 0; hi: core 7's pad row.
        bins = np.empty((NWP, K, P), np.int64)
        bins[:, 0:K1, :] = PSH - 1
        bins[:, K1:K, :] = NC * PSH - 1 - HALF
        bins[w, k, d] = val
        idx_cols = [_wrap_blocks(bins[wi].reshape(-1), splits)
                    for wi in range(NWP)]
        idx_dram = np.ascontiguousarray(np.concatenate(idx_cols, axis=1))
        per_core.append(idx_dram)
    return per_core


def build_kernel(K1, K2, reps=1):
    K = K1 + K2
    ICOL = K * 8          # idx cols per window
    splits1 = _gather_splits(K1)
    splits2 = _gather_splits(K2)

    nc = bacc.Bacc("TRN2", target_bir_lowering=False, debug=False,
                   num_swdge_queues=1)

    xT = nc.dram_tensor("xT", [P, PSH], F32, kind="ExternalInput")
    W_ext1 = nc.dram_tensor("W_ext1", [P, W1O], F32, kind="ExternalInput")
    W_ext2 = nc.dram_tensor("W_ext2", [64, W2O], F32, kind="ExternalInput")
    b1m = nc.dram_tensor("b1m", [P, 64], F32, kind="ExternalInput")
    b2m = nc.dram_tensor("b2m", [P, 40], F32, kind="ExternalInput")
    ident_in = nc.dram_tensor("ident_in", [P, P], F32, kind="ExternalInput")
    idx_in = nc.dram_tensor("idx_in", [P, NWP * ICOL], I16, kind="ExternalInput")
    out = nc.dram_tensor("out", [PSH, 40], F32, kind="ExternalOutput")

    t1_shard = nc.dram_tensor("t1_shard", [PSH, T1W], F32)
    t2_shard = nc.dram_tensor("t2_shard", [PSH, T2W], F32)
    t1 = nc.dram_tensor("t1", [NC * PSH, T1W], F32, addr_space="Shared")
    t2 = nc.dram_tensor("t2", [NC * PSH, T2W], F32, addr_space="Shared")

    with tile.TileContext(nc) as tc:
        nc.gpsimd.load_library(library_config.mlp)
        cp = tc.alloc_tile_pool(name="const", bufs=1)
        w1_sb = cp.tile([P, W1O], F32)
        nc.sync.dma_start(out=w1_sb[:], in_=W_ext1[:])
        w2_sb = cp.tile([64, W2O], F32)
        nc.sync.dma_start(out=w2_sb[:], in_=W_ext2[:])
        b1_sb = cp.tile([P, 64], F32)
        nc.sync.dma_start(out=b1_sb[:], in_=b1m[:])
        b2_sb = cp.tile([P, 40], F32)
        nc.sync.dma_start(out=b2_sb[:], in_=b2m[:])
        ident_sb = cp.tile([P, P], F32)
        nc.sync.dma_start(out=ident_sb[:], in_=ident_in[:])
        pois = cp.tile([1, 16], F32)
        nc.vector.memset(pois[:], MASKVAL)

        # two tile sets (double-buffering across loop iterations: set 1's
        # gathers/DMAs overlap set 0's compute, so cross-engine waits are
        # usually pre-satisfied when reached)
        wp = tc.alloc_tile_pool(name="work", bufs=1)
        pp = tc.alloc_tile_pool(name="ps", bufs=1, space="PSUM")
        gi2 = wp.tile([P, 2 * ICOL], I16)   # pair-batched idx staging
        xc2 = wp.tile([P, 2 * P], F32)      # pair-batched A1 input staging
        S = []
        for s in range(2):
            t = {}
            t["xc"] = wp.tile([P, P], F32, name=f"xc{s}")
            t["hb"] = wp.tile([P, W1O], F32, name=f"hb{s}")
            t["adg"] = wp.tile([P, 16], F32, name=f"adg{s}")
            t["g"] = wp.tile([P, K * T1W], F32, name=f"g{s}")
            t["ee"] = wp.tile([P, K, 8], F32, name=f"ee{s}")
            t["ex"] = wp.tile([P, K, 8], F32, name=f"ex{s}")
            t["msg"] = wp.tile([P, K, 64], F32, name=f"msg{s}")
            t["U"] = wp.tile([P, 64], F32, name=f"U{s}")
            t["den"] = wp.tile([P, 8], F32, name=f"den{s}")
            t["rec"] = wp.tile([P, 8], F32, name=f"rec{s}")
            t["agg"] = wp.tile([P, 64], F32, name=f"agg{s}")
            t["em"] = wp.tile([P, 64], F32, name=f"em{s}")
            t["h1"] = wp.tile([P, 64], F32, name=f"h1_{s}")
            t["h1c"] = wp.tile([P, 64], F32, name=f"h1c{s}")
            t["hT"] = wp.tile([64, P], F32, name=f"hT{s}")
            t["h2b"] = wp.tile([P, W2O], F32, name=f"h2b{s}")
            t["ob"] = wp.tile([P, 40], F32, name=f"ob{s}")
            t["ps1"] = pp.tile([P, W1O], F32, space="PSUM", name=f"ps1_{s}")
            t["psT"] = pp.tile([64, P], F32, space="PSUM", name=f"psT{s}")
            t["ps2"] = pp.tile([P, W2O], F32, space="PSUM", name=f"ps2_{s}")
            S.append(t)

        def a1_body(t, xc_ap, i_row, rows):
            nc.tensor.matmul(out=t["ps1"][0:rows, :], lhsT=xc_ap[:, 0:rows],
                             rhs=w1_sb[:], start=True, stop=True)
            nc.scalar.activation(out=t["hb"][0:rows, :], in_=t["ps1"][0:rows, :],
                                 func=AF.Copy)
            nc.sync.dma_start(out=t1_shard[i_row, 0:W1O], in_=t["hb"][0:rows, :])

        def edge_body(t, i_row, gi, rows, table, adtab, tshape,
                      usedw, NH, OD, post):
            """One window of the edge phase. i_row is a ds() slice; gi is
            this window's [P, ICOL] half of the pair-batched idx tile."""
            HC = NH * OD
            acol = usedw - 2 * NH
            adg = t["adg"]
            ee, ex, msg = t["ee"], t["ex"], t["msg"]
            U, den, rec, agg = t["U"], t["den"], t["rec"], t["agg"]
            gw = t["g"][:].rearrange("p (k w) -> p k w", w=tshape)[:, 0:K, :]
            nc.sync.dma_start(out=adg[0:rows, 0:2 * NH],
                              in_=adtab[i_row, acol:acol + 2 * NH])
            off = 0
            coloff = 0
            for base, n_list in ((0, splits1), (1, splits2)):
                tab_ap = table[0:HALF, :] if base == 0 else table[HALF:NC * PSH, :]
                for n_idx in n_list:
                    nc.gpsimd.dma_gather(
                        out_ap=gw[:, off:off + n_idx // P, :],
                        in_ap=tab_ap,
                        idxs_ap=gi[:, coloff:coloff + n_idx // 16],
                        num_idxs=n_idx, num_idxs_reg=n_idx, elem_size=tshape,
                        queue_num=0)
                    off += n_idx // P
                    coloff += n_idx // 16
            # logits: e = alpha_src[slot] + alpha_dst[d] + mask
            nc.vector.tensor_add(
                out=ee[:, :, 0:NH], in0=gw[:, :, acol:acol + NH],
                in1=adg[:, None, NH:2 * NH].to_broadcast([P, K, NH]))
            nc.vector.scalar_tensor_tensor(
                out=ee[:, :, 0:NH], in0=ee[:, :, 0:NH], scalar=0.2,
                in1=ee[:, :, 0:NH], op0=OP.mult, op1=OP.max)
            nc.scalar.activation(out=ex[:, :, 0:NH], in_=ee[:, :, 0:NH],
                                 func=AF.Exp)
            # msg = h[slot] * ex ; U/den = reduce over k ; agg = U/den
            nc.vector.tensor_tensor(
                out=msg[:, :, 0:HC].rearrange("p k (h o) -> p k h o", o=OD),
                in0=gw[:, :, 0:HC].rearrange("p k (h o) -> p k h o", o=OD),
                in1=ex[:, :, 0:NH, None].to_broadcast([P, K, NH, OD]),
                op=OP.mult)
            nc.vector.reduce_sum(
                out=U[:, 0:HC, None],
                in_=msg[:, :, 0:HC].rearrange("p k f -> p f k"),
                axis=mybir.AxisListType.X)
            nc.vector.reduce_sum(
                out=den[:, 0:NH, None],
                in_=ex[:, :, 0:NH].rearrange("p k h -> p h k"),
                axis=mybir.AxisListType.X)
            nc.vector.reciprocal(rec[:, 0:NH], den[:, 0:NH])
            nc.vector.tensor_tensor(
                out=agg[:, 0:HC].rearrange("p (h o) -> p h o", o=OD),
                in0=U[:, 0:HC].rearrange("p (h o) -> p h o", o=OD),
                in1=rec[:, 0:NH, None].to_broadcast([P, NH, OD]), op=OP.mult)
            post(t, rows)

        def post1(i_h1):
            def post(t, rows):
                agg, em, h1 = t["agg"], t["em"], t["h1"]
                nc.vector.tensor_add(out=agg[:, 0:64], in0=agg[:, 0:64],
                                     in1=b1_sb[:])
                nc.scalar.activation(out=em[:], in_=agg[:, 0:64], func=AF.Exp)
                nc.vector.tensor_scalar(out=em[:], in0=em[:], scalar1=-1.0,
                                        scalar2=0.0, op0=OP.add, op1=OP.min)
                nc.vector.scalar_tensor_tensor(
                    out=h1[:], in0=agg[:, 0:64], scalar=0.0, in1=em[:],
                    op0=OP.max, op1=OP.add)
                # fused layer-2 row computation: t2_shard = h1 @ W_ext2
                nc.tensor.transpose(out=t["psT"][:], in_=h1[:],
                                    identity=ident_sb[:])
                nc.scalar.activation(out=t["hT"][:], in_=t["psT"][:],
                                     func=AF.Copy)
                nc.tensor.matmul(out=t["ps2"][:], lhsT=t["hT"][:], rhs=w2_sb[:],
                                 start=True, stop=True)
                nc.scalar.activation(out=t["h2b"][:], in_=t["ps2"][:],
                                     func=AF.Copy)
                nc.sync.dma_start(out=t2_shard[i_h1, 0:W2O],
                                  in_=t["h2b"][0:rows, :])
            return post

        def post2(i_out):
            def post(t, rows):
                agg, em, ob = t["agg"], t["em"], t["ob"]
                den, rec = t["den"], t["rec"]
                nc.vector.tensor_add(out=em[:, 0:40], in0=agg[:, 0:40],
                                     in1=b2_sb[:])
                nc.scalar.activation(out=ob[:], in_=em[:, 0:40], func=AF.Exp)
                nc.vector.reduce_sum(out=den[:, 1:2, None], in_=ob[:, None, :],
                                     axis=mybir.AxisListType.X)
                nc.scalar.activation(out=rec[:, 0:1], in_=den[:, 1:2],
                                     func=AF.Ln)
                nc.vector.tensor_sub(out=ob[:], in0=em[:, 0:40],
                                     in1=rec[:, 0:1].to_broadcast([P, 40]))
                nc.sync.dma_start(out=out[i_out, :], in_=ob[0:rows, :])
            return post

        for rep in range(reps):
            # ---- A1 ----
            with tc.For_i(0, NWP * P, 2 * P) as i:
                nc.sync.dma_start(out=xc2[:], in_=xT[:, ds(i, 2 * P)])
                a1_body(S[0], xc2[:, 0:P], ds(i, P), P)
                a1_body(S[1], xc2[:, P:2 * P], ds(i + P, P), P)
            nc.sync.dma_start(out=t1_shard[PSH - 1:PSH, W1O - 16:W1O],
                              in_=pois[:, 0:16])
            nc.gpsimd.collective_compute(
                "AllGather", OP.bypass, replica_groups=[list(range(NC))],
                ins=[t1_shard[:]], outs=[t1[:]])

            # ---- B1 (layer-2 row compute fused into post1) ----
            with tc.For_i(0, NWP, 2) as i:
                nc.sync.dma_start(out=gi2[:], in_=idx_in[:, ds(i * ICOL, 2 * ICOL)])
                edge_body(S[0], ds(i * P, P), gi2[:, 0:ICOL],
                          P, t1, t1_shard, T1W, W1O, 8, 8, post1(ds(i * P, P)))
                edge_body(S[1], ds(i * P + P, P), gi2[:, ICOL:2 * ICOL],
                          P, t1, t1_shard, T1W, W1O, 8, 8,
                          post1(ds(i * P + P, P)))
            nc.sync.dma_start(out=t2_shard[PSH - 1:PSH, W2O - 2:W2O],
                              in_=pois[:, 0:2])
            nc.gpsimd.collective_compute(
                "AllGather", OP.bypass, replica_groups=[list(range(NC))],
                ins=[t2_shard[:]], outs=[t2[:]])

            # ---- B2 ----
            with tc.For_i(0, NWP, 2) as i:
                nc.sync.dma_start(out=gi2[:], in_=idx_in[:, ds(i * ICOL, 2 * ICOL)])
                edge_body(S[0], ds(i * P, P), gi2[:, 0:ICOL],
                          P, t2, t2_shard, T2W, W2O, 1, 40, post2(ds(i * P, P)))
                edge_body(S[1], ds(i * P + P, P), gi2[:, ICOL:2 * ICOL],
                          P, t2, t2_shard, T2W, W2O, 1, 40,
                          post2(ds(i * P + P, P)))

        pp.release()
        wp.release()
        cp.release()

    nc.compile()
    return nc


_CACHE = {}


def _get_nc(T, reps=1):
    key = (T, reps)
    if key not in _CACHE:
        K1, K2 = T
        _CACHE[key] = build_kernel(K1, K2, reps=reps)
    return _CACHE[key]


def make_in_maps(x, edge_index, W1, a1_src, a1_dst, b1, W2, a2_src, a2_dst, b2,
                 T, N=None):
    K1, K2 = T
    W_ext1, W_ext2 = _fold_params(W1, a1_src, a1_dst, W2, a2_src, a2_dst)
    src = np.asarray(edge_index[0]).astype(np.int64)
    dst = np.asarray(edge_index[1]).astype(np.int64)
    per_core = _prep_edges(src, dst, K1, K2)
    xTf = np.ascontiguousarray(np.asarray(x, np.float32).T)
    shared = {
        "W_ext1": W_ext1, "W_ext2": W_ext2,
        "b1m": np.tile(np.asarray(b1, np.float32)[None, :], (P, 1)),
        "b2m": np.tile(np.asarray(b2, np.float32)[None, :], (P, 1)),
        "ident_in": np.eye(P, dtype=np.float32),
    }
    maps = []
    for c, ix in enumerate(per_core):
        xp = np.zeros((P, PSH), np.float32)
        xp[:, 0:SHARD] = xTf[:, c * SHARD:(c + 1) * SHARD]
        maps.append(dict(shared, idx_in=ix, xT=xp))
    return maps


def kernel(x, edge_index, W1, a1_src, a1_dst, b1, W2, a2_src, a2_dst, b2,
           reps=1, nc_override=None):
    x = np.asarray(x, np.float32)
    edge_index = np.asarray(edge_index)
    args = [np.asarray(a, np.float32) for a in
            (W1, a1_src, a1_dst, b1, W2, a2_src, a2_dst, b2)]
    T = required_T(edge_index)
    in_maps = make_in_maps(x, edge_index, *args, T)
    nc = nc_override if nc_override is not None else _get_nc(T, reps)
    res = run_bass_kernel_spmd(nc, in_maps, list(range(NC)))
    return np.concatenate([res.results[c]["out"][0:SHARD] for c in range(NC)],
                          axis=0)
